# revision 15
# baseline (speedup 1.0000x reference)
"""Trainium2 Bass kernel for nn_CascadeLayer (gnn_message_passing).

Cascade of GegConv layers, K = 1..4, alpha = 0.5, lambda_max = 2.0.
Since 2/lambda_max == 1, lhat(h) == -prop(h), so the whole cascade is
three sparse propagates plus small dense matmuls:

    P1 = prop(x)    Tx1 = -P1
    P2 = prop(Tx1)  Tx2 = -1.5*P2 - 0.5*x
    P3 = prop(Tx2)  Tx3 = -(5/3)*P3 - (2/3)*Tx1
    out_i = relu(sum_k Tx_k @ W_i[k] + b_i)

Distribution: nodes sharded across 8 cores by range (graph parallel).
Edges are routed to the core owning their destination. Every edge goes
through the gather path: per-edge 256B bf16 source rows are fetched with
dma_gather from a local DRAM replica of the full activation (kept in
sync with an AllGather between stages), scaled by the precomputed
symmetric norm on the VectorEngine, and segment-summed per destination
via an ELL layout (lane=partition, slot=chunk) with a strided
tensor_reduce. Lanes are degree-sorted into 128-lane groups; groups are
packed into NB contiguous batches with balanced slot counts, and each
batch issues one big gather per source window (A = cores 0-4, B =
cores 3-7; two windows keep replica rows int16-addressable).
"""

import numpy as np

import concourse.bass as bass
import concourse.bacc as bacc
import concourse.mybir as mybir
import concourse.tile as tile
from concourse import bass_utils

NCORES = 8
N = 50000
E = 800000
D = 128
SH = 6250                 # real nodes per core shard
LANES = 6272              # padded lanes per core (49 * 128)
G = LANES // 128          # 49 lane groups
SHARD_ROWS = LANES + 1    # + zero row for padding gathers
TOT_ROWS = NCORES * SHARD_ROWS          # 50184 replica rows
WIN_A = (0, 5 * SHARD_ROWS)             # replica rows of cores 0-4 (31365)
WIN_B = (3 * SHARD_ROWS, TOT_ROWS)      # replica rows of cores 3-7 (31365)
A_SCALE = (-1.0, -1.5, -5.0 / 3.0)      # prop scale folded into w per stage
B_SCALE = (0.0, -0.5, -2.0 / 3.0)       # partner scale per stage
NB = 10                   # balanced batches per stage

F32 = mybir.dt.float32
BF16 = mybir.dt.bfloat16
I16 = mybir.dt.int16


def _preprocess(edge_index, edge_weight):
    """Build per-core gather/scale structures."""
    row = edge_index[0].astype(np.int64)
    col = edge_index[1].astype(np.int64)
    ew = np.asarray(edge_weight, np.float32)

    deg = np.zeros(N, np.float32)
    np.add.at(deg, row, ew)
    dis = np.where(deg > 0, 1.0 / np.sqrt(np.where(deg > 0, deg, 1.0)), 0.0)
    dis = dis.astype(np.float32)
    norm = (dis[row].astype(np.float64) * ew * dis[col]).astype(np.float32)

    core_of_node = np.minimum(np.arange(N) // SH, NCORES - 1)
    src_core = core_of_node[row]
    dst_core = core_of_node[col]

    # --- per-core lane ordering (degree snake-sort) -----------------------
    # hard0: src core 0-2 (must use window A), hard1: src core 5-7 (B),
    # free: src core 3-4 (either window).
    nodes_sorted = np.zeros((NCORES, LANES), np.int64)
    per_core = []
    lane_lo = np.zeros((NCORES, LANES), np.int64)   # hard0 count
    lane_hi = np.zeros((NCORES, LANES), np.int64)   # hard0 + free
    lane_tot = np.zeros((NCORES, LANES), np.int64)
    for c in range(NCORES):
        em = dst_core == c
        d_loc = col[em] - c * SH
        sc = src_core[em]
        h0 = np.bincount(d_loc[sc <= 2], minlength=SH)
        h1 = np.bincount(d_loc[sc >= 5], minlength=SH)
        fr = np.bincount(d_loc[(sc == 3) | (sc == 4)], minlength=SH)
        tot = h0 + h1 + fr
        # extend with dummy lanes (degree 0)
        tot_e = np.concatenate([tot, np.zeros(LANES - SH, np.int64)])
        sk_e = np.concatenate([h0 - h1, np.zeros(LANES - SH, np.int64)])
        h0_e = np.concatenate([h0, np.zeros(LANES - SH, np.int64)])
        fr_e = np.concatenate([fr, np.zeros(LANES - SH, np.int64)])
        ids_e = np.concatenate([np.arange(SH, dtype=np.int64) + c * SH,
                                np.full(LANES - SH, -1, np.int64)])
        # snake-sort by (tot, +-skew)
        key = np.where(tot_e % 2 == 0, sk_e, -sk_e)
        o = np.lexsort((key, tot_e))
        nodes_sorted[c] = ids_e[o]
        lane_lo[c] = h0_e[o]
        lane_hi[c] = h0_e[o] + fr_e[o]
        lane_tot[c] = tot_e[o]
        per_core.append(em)

    # joint capacity choice per (sorted-order) group: same for all cores
    S0g = np.zeros(G, np.int64)
    S1g = np.zeros(G, np.int64)
    for g in range(G):
        sl = slice(g * 128, (g + 1) * 128)
        LO = lane_lo[:, sl]; HI = lane_hi[:, sl]; T = lane_tot[:, sl]
        smin = int(LO.max())
        best = None
        for s0 in range(smin, smin + 64):
            s1 = int((T - np.minimum(HI, s0)).max())
            if best is None or s0 + s1 < best[0]:
                best = (s0 + s1, s0, s1)
        S0g[g], S1g[g] = best[1], best[2]

    # --- balanced batch assignment (LPT), then relabel lanes --------------
    sizes = S0g + S1g
    order = np.argsort(sizes)[::-1]
    cap = [G // NB + (1 if b < G % NB else 0) for b in range(NB)]
    batch_groups = [[] for _ in range(NB)]
    batch_load = np.zeros(NB, np.int64)
    for g in order:
        cands = [b for b in range(NB) if len(batch_groups[b]) < cap[b]]
        b = min(cands, key=lambda bb: batch_load[bb])
        batch_groups[b].append(int(g))
        batch_load[b] += sizes[g]
    new_order = [g for b in range(NB) for g in batch_groups[b]]
    # permute per-group arrays and lane-level arrays to the new order
    lane_perm = np.concatenate([np.arange(g * 128, (g + 1) * 128)
                                for g in new_order])
    S0n = S0g[new_order]
    S1n = S1g[new_order]
    nodes_sorted = nodes_sorted[:, lane_perm]
    lane_lo = lane_lo[:, lane_perm]
    lane_hi = lane_hi[:, lane_perm]
    lane_tot = lane_tot[:, lane_perm]

    lane_of_node = np.full(N, -1, np.int64)
    node_of_lane = np.full((NCORES, LANES), -1, np.int64)
    perm_row = np.zeros(N, np.int64)
    for c in range(NCORES):
        nodes = nodes_sorted[c]
        node_of_lane[c] = nodes
        real = nodes >= 0
        lane_of_node[nodes[real]] = np.nonzero(real)[0]
        perm_row[nodes[real]] = c * SHARD_ROWS + np.nonzero(real)[0]

    # per-core per-lane split: x = lane's window-A count
    xA = np.maximum(lane_lo, lane_tot - np.repeat(S1n, 128)[None, :])

    # --- batch/call structure (static, same for all cores) ----------------
    # global chunk sequence: [b0.A | b0.B | b1.A | b1.B | ...]
    batches = []
    g0 = 0
    for b in range(NB):
        nbg = len(batch_groups[b])
        batches.append(list(range(g0, g0 + nbg)))
        g0 += nbg
    calls = []          # per batch: dict with A/B info
    col_off = 0
    w_off = 0
    chunk_base = np.zeros((G, 2), np.int64)
    for bi, gs in enumerate(batches):
        info = {"groups": gs, "lane0": gs[0] * 128,
                "nlanes": len(gs) * 128}
        for w, Sn in ((0, S0n), (1, S1n)):
            nch = int(sum(Sn[g] for g in gs))
            o = 0
            offs = {}
            for g in gs:
                offs[g] = o
                chunk_base[g, w] = w_off + o
                o += int(Sn[g])
            info[f"nch{w}"] = nch
            info[f"coff{w}"] = col_off
            info[f"woff{w}"] = w_off
            info[f"offs{w}"] = offs
            col_off += nch * 128 // 16
            w_off += nch
        calls.append(info)
    total_cols = col_off
    total_chunks = w_off

    # --- fill idx / w slot arrays per core --------------------------------
    idx_arrs = np.zeros((NCORES, 128, total_cols), np.int16)
    w_base = np.zeros((NCORES, 128, total_chunks), np.float32)
    for c in range(NCORES):
        em = per_core[c]
        e_lane = lane_of_node[col[em]]
        e_src_row = perm_row[row[em]]
        e_norm = norm[em]
        e_free = (src_core[em] == 3) | (src_core[em] == 4)
        e_hard1 = src_core[em] >= 5
        # order edges per lane: hard0 first, then free, then hard1
        cls = np.where(e_hard1, 2, np.where(e_free, 1, 0))
        eo = np.lexsort((cls, e_lane))
        e_lane, e_src_row, e_norm = e_lane[eo], e_src_row[eo], e_norm[eo]
        # position within lane
        lane_start = np.zeros(LANES + 1, np.int64)
        np.add.at(lane_start, e_lane + 1, 1)
        lane_start = np.cumsum(lane_start)
        pos_in_lane = np.arange(len(e_lane)) - lane_start[e_lane]
        # window: first xA[lane] edges -> A, rest -> B
        in_a = pos_in_lane < xA[c][e_lane]
        slot = np.where(in_a, pos_in_lane, pos_in_lane - xA[c][e_lane])
        widx = np.where(in_a, e_src_row - WIN_A[0], e_src_row - WIN_B[0])
        assert widx.min() >= 0
        assert widx.max() < 32768

        e_grp = e_lane // 128
        e_li = e_lane % 128
        e_chunk = chunk_base[e_grp, np.where(in_a, 0, 1)] + slot
        flat_idx = np.full(total_chunks * 128, -1, np.int64)
        flat_w = np.zeros(total_chunks * 128, np.float32)
        p = e_chunk * 128 + e_li
        assert len(np.unique(p)) == len(p)
        flat_idx[p] = widx
        flat_w[p] = e_norm
        # pads -> zero row of the window (row 6272 relative to window base)
        flat = flat_idx.reshape(total_chunks, 128)
        flat[flat < 0] = SHARD_ROWS - 1
        # wrapped layout: wrapped[q, col] = L[col*16 + q]
        wrapped = flat_idx.reshape(total_cols, 16).T.astype(np.int16)
        idx_arrs[c] = np.tile(wrapped, (8, 1))
        w_base[c] = flat_w.reshape(total_chunks, 128).T  # [li, chunk]

    meta = dict(S0n=S0n, S1n=S1n, calls=calls, batches=batches,
                total_cols=total_cols, total_chunks=total_chunks,
                node_of_lane=node_of_lane)
    return idx_arrs, w_base, perm_row, meta


def _build_program(meta):
    total_cols = meta["total_cols"]
    total_chunks = meta["total_chunks"]
    calls = meta["calls"]
    S0n, S1n = meta["S0n"], meta["S1n"]

    nc = bacc.Bacc("TRN2", target_bir_lowering=False, debug=False,
                   num_devices=NCORES, num_swdge_queues=4)

    xr = nc.dram_tensor("xr", [TOT_ROWS, D], BF16, kind="ExternalInput")
    x_fm = nc.dram_tensor("x_fm", [128, LANES], BF16, kind="ExternalInput")
    x_nm_d = nc.dram_tensor("x_nm", [128, LANES], BF16, kind="ExternalInput")
    idx_d = nc.dram_tensor("idx", [128, total_cols], I16, kind="ExternalInput")
    w_d = [nc.dram_tensor(f"w{k}", [128, total_chunks], BF16, kind="ExternalInput")
           for k in range(3)]
    W_d = [nc.dram_tensor(f"W{i+1}", [i + 1, D, D], BF16, kind="ExternalInput")
           for i in range(4)]
    onesb_d = nc.dram_tensor("onesb", [D, D], BF16, kind="ExternalInput")
    bias_d = [nc.dram_tensor(f"bias{i+1}", [D, D], BF16, kind="ExternalInput")
              for i in range(4)]
    out_d = [nc.dram_tensor(f"o{i+1}", [LANES, D], F32, kind="ExternalOutput")
             for i in range(4)]

    with tile.TileContext(nc) as tc:
        with (
            tc.tile_pool(name="pers", bufs=1) as pers,
            tc.tile_pool(name="msgs", bufs=3) as msgs_pool,
            tc.tile_pool(name="work", bufs=4) as work,
            tc.tile_pool(name="txbp", bufs=2) as txbp,
            tc.tile_pool(name="outp", bufs=3) as outp,
            tc.tile_pool(name="pt", bufs=2, space="PSUM") as pt,
            tc.tile_pool(name="pd", bufs=2, space="PSUM") as pd,
            tc.tile_pool(name="pr", bufs=3, space="PSUM") as pr,
            tc.tile_pool(name="dram", bufs=1, space="DRAM") as dram,
        ):
            # ---------------- prologue ----------------
            idx_t = pers.tile([128, total_cols], I16, tag="idx", name="idx_t")
            nc.sync.dma_start(out=idx_t[:], in_=idx_d[:])
            w_t = [pers.tile([128, total_chunks], BF16, tag=f"w{k}",
                             name=f"w_t{k}") for k in range(3)]
            for k in range(3):
                nc.sync.dma_start(out=w_t[k][:], in_=w_d[k][:])
            x_nm = pers.tile([128, LANES], BF16, tag="x_nm", name="x_nm")
            nc.sync.dma_start(out=x_nm[:], in_=x_nm_d[:])
            txT = [pers.tile([128, LANES], BF16, tag=f"txT{k}",
                             name=f"txT{k}") for k in range(4)]
            nc.sync.dma_start(out=txT[0][:], in_=x_fm[:])
            tx1_nm = pers.tile([128, LANES], BF16, tag="tx1_nm", name="tx1_nm")
            W_t = []          # W_t[i][k]: [cin, cout] bf16
            for i in range(4):
                tiles = []
                for k in range(i + 1):
                    wt = pers.tile([D, D], BF16, tag=f"W{i}{k}", name=f"W_t{i}{k}")
                    nc.sync.dma_start(out=wt[:], in_=W_d[i][k])
                    tiles.append(wt)
                W_t.append(tiles)
            onesb = pers.tile([D, D], BF16, tag="onesb", name="onesb_t")
            nc.sync.dma_start(out=onesb[:], in_=onesb_d[:])
            bias_t = []
            for i in range(4):
                bt = pers.tile([D, D], BF16, tag=f"bias{i}", name=f"bias_t{i}")
                nc.sync.dma_start(out=bt[:], in_=bias_d[i][:])
                bias_t.append(bt)
            ident = pers.tile([128, 128], BF16, tag="ident", name="ident")
            from concourse.masks import make_identity
            make_identity(nc, ident[:])
            zero_b = pers.tile([128, D], BF16, tag="zerob", name="zero_b")
            nc.gpsimd.memset(zero_b[:], 0.0)

            # DRAM: AG bounces + replicas
            bounce = [dram.tile([SHARD_ROWS, D], BF16, tag=f"bounce{k}",
                                name=f"bounce{k}") for k in range(2)]
            repl = [dram.tile([TOT_ROWS, D], BF16, tag=f"repl{k}",
                              name=f"repl{k}", addr_space="Shared")
                    for k in range(2)]
            for k in range(2):
                nc.sync.dma_start(out=bounce[k][SHARD_ROWS - 1:SHARD_ROWS, :],
                                  in_=zero_b[0:1, :])

            def dense_tile(i, g):
                ps = pd.tile([128, 128], F32, tag="pdt", name="pdt")
                nc.tensor.matmul(out=ps[:], lhsT=onesb[:], rhs=bias_t[i][:],
                                 start=True, stop=False)
                for k in range(i + 1):
                    nc.tensor.matmul(out=ps[:],
                                     lhsT=txT[k][:, g * 128:(g + 1) * 128],
                                     rhs=W_t[i][k][:],
                                     start=False, stop=(k == i))
                ot = outp.tile([128, D], F32, tag="ot", name="ot")
                nc.scalar.activation(out=ot[:], in_=ps[:],
                                     func=mybir.ActivationFunctionType.Relu)
                nc.sync.dma_start(out=out_d[i][g * 128:(g + 1) * 128, :],
                                  in_=ot[:])

            for g in range(G):
                dense_tile(0, g)

            def stage(k):
                """k = 0,1,2 computes Tx_{k+1}; gathers from src replica."""
                src = xr if k == 0 else repl[k - 1]
                winA = src[WIN_A[0]:WIN_A[1], :]
                winB = src[WIN_B[0]:WIN_B[1], :]
                wk = w_t[k]
                qrr = [0]

                for bi, info in enumerate(calls):
                    gs = info["groups"]
                    nchA, nchB = info["nch0"], info["nch1"]
                    nch = nchA + nchB
                    m = msgs_pool.tile([128, nch, D], BF16, tag="m", name="m")
                    for w, win in ((0, winA), (1, winB)):
                        nw = info[f"nch{w}"]
                        if nw == 0:
                            continue
                        o0 = 0 if w == 0 else nchA
                        # split across SWDGE queues 1/2: desc-gen on queue 0
                        # runs synchronously on the Pool engine, while
                        # queues >= 1 generate asynchronously on separate
                        # Q7 workers — engine retires in ~1us
                        h0 = nw // 2
                        for c0, cn in ((0, h0), (h0, nw - h0)):
                            if cn == 0:
                                continue
                            q = 1 + (qrr[0] % 3)
                            qrr[0] += 1
                            nc.gpsimd.dma_gather(
                                out_ap=m[:, o0 + c0:o0 + c0 + cn, :],
                                in_ap=win,
                                idxs_ap=idx_t[:, info[f"coff{w}"] + c0 * 8:
                                              info[f"coff{w}"] + (c0 + cn) * 8],
                                num_idxs=cn * 128,
                                num_idxs_reg=cn * 128,
                                elem_size=D,
                                single_packet=False,
                                queue_num=q,
                            )
                        # scale by w (broadcast along feat)
                        nc.vector.tensor_tensor(
                            out=m[:, o0:o0 + nw, :],
                            in0=m[:, o0:o0 + nw, :],
                            in1=wk[:, info[f"woff{w}"]:info[f"woff{w}"] + nw]
                                .unsqueeze(2).broadcast_to([128, nw, D]),
                            op=mybir.AluOpType.mult,
                        )
                    txb = txbp.tile([128, len(gs) * 128], BF16, tag="txb",
                                    name="txb")
                    for gi, g in enumerate(gs):
                        gsl = slice(g * 128, (g + 1) * 128)
                        s0, s1 = int(S0n[g]), int(S1n[g])
                        oA = info["offs0"][g]
                        oB = nchA + info["offs1"][g]
                        parts = [(oA, s0), (oB, s1)]
                        parts = [(o, s) for (o, s) in parts if s > 0]
                        assert parts
                        # segment-sum on the TensorEngine: accumulate the
                        # group's chunks into PSUM via identity matmuls
                        P = pr.tile([128, 128], F32, tag="pr", name="pr")
                        seq = [o + j for (o, s) in parts for j in range(s)]
                        for ji, c in enumerate(seq):
                            nc.tensor.matmul(out=P[:], lhsT=ident[:],
                                             rhs=m[:, c, :],
                                             start=(ji == 0),
                                             stop=(ji == len(seq) - 1))
                        # recurrence: Tx_{k+1} = A + B_SCALE * partner
                        if k > 0:
                            partner = x_nm if k == 1 else tx1_nm
                            nc.vector.scalar_tensor_tensor(
                                out=P[:],
                                in0=partner[:, gsl],
                                scalar=float(B_SCALE[k]),
                                in1=P[:],
                                op0=mybir.AluOpType.mult,
                                op1=mybir.AluOpType.add,
                            )
                        # bf16 node-major copy (bounce staging + transpose src)
                        nc.scalar.copy(out=txb[:, gi * 128:(gi + 1) * 128],
                                       in_=P[:])
                        if k == 0:
                            nc.scalar.copy(out=tx1_nm[:, gsl],
                                           in_=txb[:, gi * 128:(gi + 1) * 128])
                        psT = pt.tile([128, 128], BF16, tag="ptt", name="ptt")
                        nc.tensor.transpose(
                            out=psT[:],
                            in_=txb[:, gi * 128:(gi + 1) * 128],
                            identity=ident[:])
                        nc.scalar.copy(out=txT[k + 1][:, gsl], in_=psT[:])
                        dense_tile(k + 1, g)
                    if k < 2:
                        r0 = info["lane0"]
                        nr = info["nlanes"]
                        nc.sync.dma_start(
                            out=bounce[k][r0:r0 + nr, :]
                                .rearrange("(j p) f -> p j f", p=128),
                            in_=txb[:].rearrange("p (j f) -> p j f", f=D))
                if k < 2:
                    nc.gpsimd.collective_compute(
                        "AllGather",
                        mybir.AluOpType.bypass,
                        replica_groups=[list(range(NCORES))],
                        ins=[bounce[k][:].opt()],
                        outs=[repl[k][:].opt()],
                    )

            stage(0)
            stage(1)
            stage(2)

    nc.compile()
    return nc


def kernel(x, edge_index, edge_weight, W1, W2, W3, W4, b1, b2, b3, b4,
           _trace=False):
    import ml_dtypes
    x = np.asarray(x, np.float32)
    edge_index = np.asarray(edge_index)
    edge_weight = np.asarray(edge_weight, np.float32)
    Ws = [np.asarray(w, np.float32) for w in (W1, W2, W3, W4)]
    bs = [np.asarray(b, np.float32) for b in (b1, b2, b3, b4)]

    idx_arrs, w_base, perm_row, meta = _preprocess(edge_index, edge_weight)
    nc = _build_program(meta)

    # replica of x in permuted layout (zero rows stay zero)
    xr = np.zeros((TOT_ROWS, D), np.float32)
    xr[perm_row] = x
    xr = xr.astype(ml_dtypes.bfloat16)
    onesb = np.zeros((D, D), np.float32); onesb[0, :] = 1.0
    in_maps = []
    for c in range(NCORES):
        nol = meta["node_of_lane"][c]
        xs_c = np.zeros((LANES, D), np.float32)
        real = nol >= 0
        xs_c[real] = x[nol[real]]
        xs_b = xs_c.astype(ml_dtypes.bfloat16)
        m = {
            "xr": xr,
            "x_fm": np.ascontiguousarray(xs_b.T),
            "x_nm": np.ascontiguousarray(
                xs_b.reshape(G, 128, D).transpose(1, 0, 2).reshape(128, G * D)),
            "idx": idx_arrs[c],
            "onesb": onesb.astype(ml_dtypes.bfloat16),
        }
        for k in range(3):
            m[f"w{k}"] = (A_SCALE[k] * w_base[c]).astype(ml_dtypes.bfloat16)
        for i in range(4):
            m[f"W{i+1}"] = Ws[i].astype(ml_dtypes.bfloat16)
            bb = np.zeros((D, D), np.float32); bb[0, :] = bs[i]
            m[f"bias{i+1}"] = bb.astype(ml_dtypes.bfloat16)
        in_maps.append(m)

    res = bass_utils.run_bass_kernel_spmd(
        nc, in_maps, core_ids=list(range(NCORES)), trace=_trace)

    outs = []
    for i in range(4):
        full = np.zeros((N, D), np.float32)
        for c in range(NCORES):
            nol = meta["node_of_lane"][c]
            real = nol >= 0
            full[nol[real]] = res.results[c][f"o{i+1}"][real]
        outs.append(full)
    if _trace:
        return tuple(outs), res
    return tuple(outs)


# revision 18
# speedup vs baseline: 1.0214x; 1.0214x over previous
"""Trainium2 Bass kernel for nn_CascadeLayer (gnn_message_passing).

Cascade of GegConv layers, K = 1..4, alpha = 0.5, lambda_max = 2.0.
Since 2/lambda_max == 1, lhat(h) == -prop(h), so the whole cascade is
three sparse propagates plus small dense matmuls:

    P1 = prop(x)    Tx1 = -P1
    P2 = prop(Tx1)  Tx2 = -1.5*P2 - 0.5*x
    P3 = prop(Tx2)  Tx3 = -(5/3)*P3 - (2/3)*Tx1
    out_i = relu(sum_k Tx_k @ W_i[k] + b_i)

Distribution: nodes sharded across 8 cores by range (graph parallel).
Edges are routed to the core owning their destination. Every edge goes
through the gather path: per-edge 256B bf16 source rows are fetched with
dma_gather from a local DRAM replica of the full activation (kept in
sync with an AllGather between stages), scaled by the precomputed
symmetric norm on the VectorEngine, and segment-summed per destination
via an ELL layout (lane=partition, slot=chunk) with a strided
tensor_reduce. Lanes are degree-sorted into 128-lane groups; groups are
packed into NB contiguous batches with balanced slot counts, and each
batch issues one big gather per source window (A = cores 0-4, B =
cores 3-7; two windows keep replica rows int16-addressable).
"""

import numpy as np

import concourse.bass as bass
import concourse.bacc as bacc
import concourse.mybir as mybir
import concourse.tile as tile
from concourse import bass_utils

NCORES = 8
N = 50000
E = 800000
D = 128
SH = 6250                 # real nodes per core shard
LANES = 6272              # padded lanes per core (49 * 128)
G = LANES // 128          # 49 lane groups
SHARD_ROWS = LANES + 1    # + zero row for padding gathers
TOT_ROWS = NCORES * SHARD_ROWS          # 50184 replica rows
WIN_A = (0, 5 * SHARD_ROWS)             # replica rows of cores 0-4 (31365)
WIN_B = (3 * SHARD_ROWS, TOT_ROWS)      # replica rows of cores 3-7 (31365)
A_SCALE = (-1.0, -1.5, -5.0 / 3.0)      # prop scale folded into w per stage
B_SCALE = (0.0, -0.5, -2.0 / 3.0)       # partner scale per stage
NB = 20                   # balanced batches per stage

F32 = mybir.dt.float32
BF16 = mybir.dt.bfloat16
I16 = mybir.dt.int16


def _preprocess(edge_index, edge_weight):
    """Build per-core gather/scale structures."""
    row = edge_index[0].astype(np.int64)
    col = edge_index[1].astype(np.int64)
    ew = np.asarray(edge_weight, np.float32)

    deg = np.zeros(N, np.float32)
    np.add.at(deg, row, ew)
    dis = np.where(deg > 0, 1.0 / np.sqrt(np.where(deg > 0, deg, 1.0)), 0.0)
    dis = dis.astype(np.float32)
    norm = (dis[row].astype(np.float64) * ew * dis[col]).astype(np.float32)

    core_of_node = np.minimum(np.arange(N) // SH, NCORES - 1)
    src_core = core_of_node[row]
    dst_core = core_of_node[col]

    # --- per-core lane ordering (degree snake-sort) -----------------------
    # hard0: src core 0-2 (must use window A), hard1: src core 5-7 (B),
    # free: src core 3-4 (either window).
    nodes_sorted = np.zeros((NCORES, LANES), np.int64)
    per_core = []
    lane_lo = np.zeros((NCORES, LANES), np.int64)   # hard0 count
    lane_hi = np.zeros((NCORES, LANES), np.int64)   # hard0 + free
    lane_tot = np.zeros((NCORES, LANES), np.int64)
    for c in range(NCORES):
        em = dst_core == c
        d_loc = col[em] - c * SH
        sc = src_core[em]
        h0 = np.bincount(d_loc[sc <= 2], minlength=SH)
        h1 = np.bincount(d_loc[sc >= 5], minlength=SH)
        fr = np.bincount(d_loc[(sc == 3) | (sc == 4)], minlength=SH)
        tot = h0 + h1 + fr
        # extend with dummy lanes (degree 0)
        tot_e = np.concatenate([tot, np.zeros(LANES - SH, np.int64)])
        sk_e = np.concatenate([h0 - h1, np.zeros(LANES - SH, np.int64)])
        h0_e = np.concatenate([h0, np.zeros(LANES - SH, np.int64)])
        fr_e = np.concatenate([fr, np.zeros(LANES - SH, np.int64)])
        ids_e = np.concatenate([np.arange(SH, dtype=np.int64) + c * SH,
                                np.full(LANES - SH, -1, np.int64)])
        # snake-sort by (tot, +-skew)
        key = np.where(tot_e % 2 == 0, sk_e, -sk_e)
        o = np.lexsort((key, tot_e))
        nodes_sorted[c] = ids_e[o]
        lane_lo[c] = h0_e[o]
        lane_hi[c] = h0_e[o] + fr_e[o]
        lane_tot[c] = tot_e[o]
        per_core.append(em)

    # joint capacity choice per (sorted-order) group: same for all cores
    S0g = np.zeros(G, np.int64)
    S1g = np.zeros(G, np.int64)
    for g in range(G):
        sl = slice(g * 128, (g + 1) * 128)
        LO = lane_lo[:, sl]; HI = lane_hi[:, sl]; T = lane_tot[:, sl]
        smin = int(LO.max())
        best = None
        for s0 in range(smin, smin + 64):
            s1 = int((T - np.minimum(HI, s0)).max())
            if best is None or s0 + s1 < best[0]:
                best = (s0 + s1, s0, s1)
        S0g[g], S1g[g] = best[1], best[2]

    # --- balanced batch assignment (LPT), then relabel lanes --------------
    sizes = S0g + S1g
    order = np.argsort(sizes)[::-1]
    cap = [-(-G // NB)] * NB
    batch_groups = [[] for _ in range(NB)]
    batch_load = np.zeros(NB, np.int64)
    for g in order:
        cands = [b for b in range(NB) if len(batch_groups[b]) < cap[b]]
        b = min(cands, key=lambda bb: batch_load[bb])
        batch_groups[b].append(int(g))
        batch_load[b] += sizes[g]
    new_order = [g for b in range(NB) for g in batch_groups[b]]
    # permute per-group arrays and lane-level arrays to the new order
    lane_perm = np.concatenate([np.arange(g * 128, (g + 1) * 128)
                                for g in new_order])
    S0n = S0g[new_order]
    S1n = S1g[new_order]
    nodes_sorted = nodes_sorted[:, lane_perm]
    lane_lo = lane_lo[:, lane_perm]
    lane_hi = lane_hi[:, lane_perm]
    lane_tot = lane_tot[:, lane_perm]

    lane_of_node = np.full(N, -1, np.int64)
    node_of_lane = np.full((NCORES, LANES), -1, np.int64)
    perm_row = np.zeros(N, np.int64)
    for c in range(NCORES):
        nodes = nodes_sorted[c]
        node_of_lane[c] = nodes
        real = nodes >= 0
        lane_of_node[nodes[real]] = np.nonzero(real)[0]
        perm_row[nodes[real]] = c * SHARD_ROWS + np.nonzero(real)[0]

    # per-core per-lane split: x = lane's window-A count
    xA = np.maximum(lane_lo, lane_tot - np.repeat(S1n, 128)[None, :])

    # --- batch/call structure (static, same for all cores) ----------------
    # global chunk sequence: [b0.A | b0.B | b1.A | b1.B | ...]
    batches = []
    g0 = 0
    for b in range(NB):
        nbg = len(batch_groups[b])
        batches.append(list(range(g0, g0 + nbg)))
        g0 += nbg
    calls = []          # per batch: dict with A/B info
    col_off = 0
    w_off = 0
    chunk_base = np.zeros((G, 2), np.int64)
    for bi, gs in enumerate(batches):
        info = {"groups": gs, "lane0": gs[0] * 128,
                "nlanes": len(gs) * 128}
        for w, Sn in ((0, S0n), (1, S1n)):
            nch = int(sum(Sn[g] for g in gs))
            o = 0
            offs = {}
            for g in gs:
                offs[g] = o
                chunk_base[g, w] = w_off + o
                o += int(Sn[g])
            info[f"nch{w}"] = nch
            info[f"coff{w}"] = col_off
            info[f"woff{w}"] = w_off
            info[f"offs{w}"] = offs
            col_off += nch * 128 // 16
            w_off += nch
        calls.append(info)
    total_cols = col_off
    total_chunks = w_off

    # --- fill idx / w slot arrays per core --------------------------------
    idx_arrs = np.zeros((NCORES, 128, total_cols), np.int16)
    w_base = np.zeros((NCORES, 128, total_chunks), np.float32)
    for c in range(NCORES):
        em = per_core[c]
        e_lane = lane_of_node[col[em]]
        e_src_row = perm_row[row[em]]
        e_norm = norm[em]
        e_free = (src_core[em] == 3) | (src_core[em] == 4)
        e_hard1 = src_core[em] >= 5
        # order edges per lane: hard0 first, then free, then hard1
        cls = np.where(e_hard1, 2, np.where(e_free, 1, 0))
        eo = np.lexsort((cls, e_lane))
        e_lane, e_src_row, e_norm = e_lane[eo], e_src_row[eo], e_norm[eo]
        # position within lane
        lane_start = np.zeros(LANES + 1, np.int64)
        np.add.at(lane_start, e_lane + 1, 1)
        lane_start = np.cumsum(lane_start)
        pos_in_lane = np.arange(len(e_lane)) - lane_start[e_lane]
        # window: first xA[lane] edges -> A, rest -> B
        in_a = pos_in_lane < xA[c][e_lane]
        slot = np.where(in_a, pos_in_lane, pos_in_lane - xA[c][e_lane])
        widx = np.where(in_a, e_src_row - WIN_A[0], e_src_row - WIN_B[0])
        assert widx.min() >= 0
        assert widx.max() < 32768

        e_grp = e_lane // 128
        e_li = e_lane % 128
        e_chunk = chunk_base[e_grp, np.where(in_a, 0, 1)] + slot
        flat_idx = np.full(total_chunks * 128, -1, np.int64)
        flat_w = np.zeros(total_chunks * 128, np.float32)
        p = e_chunk * 128 + e_li
        assert len(np.unique(p)) == len(p)
        flat_idx[p] = widx
        flat_w[p] = e_norm
        # pads -> zero row of the window (row 6272 relative to window base)
        flat = flat_idx.reshape(total_chunks, 128)
        flat[flat < 0] = SHARD_ROWS - 1
        # wrapped layout: wrapped[q, col] = L[col*16 + q]
        wrapped = flat_idx.reshape(total_cols, 16).T.astype(np.int16)
        idx_arrs[c] = np.tile(wrapped, (8, 1))
        w_base[c] = flat_w.reshape(total_chunks, 128).T  # [li, chunk]

    meta = dict(S0n=S0n, S1n=S1n, calls=calls, batches=batches,
                total_cols=total_cols, total_chunks=total_chunks,
                node_of_lane=node_of_lane)
    return idx_arrs, w_base, perm_row, meta


def _build_program(meta):
    total_cols = meta["total_cols"]
    total_chunks = meta["total_chunks"]
    calls = meta["calls"]
    S0n, S1n = meta["S0n"], meta["S1n"]

    nc = bacc.Bacc("TRN2", target_bir_lowering=False, debug=False,
                   num_devices=NCORES, num_swdge_queues=4)

    xr = nc.dram_tensor("xr", [TOT_ROWS, D], BF16, kind="ExternalInput")
    x_fm = nc.dram_tensor("x_fm", [128, LANES], BF16, kind="ExternalInput")
    x_nm_d = nc.dram_tensor("x_nm", [128, LANES], BF16, kind="ExternalInput")
    idx_d = nc.dram_tensor("idx", [128, total_cols], I16, kind="ExternalInput")
    w_d = [nc.dram_tensor(f"w{k}", [128, total_chunks], BF16, kind="ExternalInput")
           for k in range(3)]
    W_d = [nc.dram_tensor(f"W{i+1}", [i + 1, D, D], BF16, kind="ExternalInput")
           for i in range(4)]
    onesb_d = nc.dram_tensor("onesb", [D, D], BF16, kind="ExternalInput")
    bias_d = [nc.dram_tensor(f"bias{i+1}", [D, D], BF16, kind="ExternalInput")
              for i in range(4)]
    out_d = [nc.dram_tensor(f"o{i+1}", [LANES, D], F32, kind="ExternalOutput")
             for i in range(4)]

    with tile.TileContext(nc) as tc:
        with (
            tc.tile_pool(name="pers", bufs=1) as pers,
            tc.tile_pool(name="msgs", bufs=5) as msgs_pool,
            tc.tile_pool(name="work", bufs=4) as work,
            tc.tile_pool(name="txbp", bufs=2) as txbp,
            tc.tile_pool(name="outp", bufs=3) as outp,
            tc.tile_pool(name="pt", bufs=2, space="PSUM") as pt,
            tc.tile_pool(name="pd", bufs=2, space="PSUM") as pd,
            tc.tile_pool(name="pr", bufs=3, space="PSUM") as pr,
            tc.tile_pool(name="dram", bufs=1, space="DRAM") as dram,
        ):
            # ---------------- prologue ----------------
            idx_t = pers.tile([128, total_cols], I16, tag="idx", name="idx_t")
            nc.sync.dma_start(out=idx_t[:], in_=idx_d[:])
            w_t = [pers.tile([128, total_chunks], BF16, tag=f"w{k}",
                             name=f"w_t{k}") for k in range(3)]
            for k in range(3):
                nc.sync.dma_start(out=w_t[k][:], in_=w_d[k][:])
            x_nm = pers.tile([128, LANES], BF16, tag="x_nm", name="x_nm")
            nc.sync.dma_start(out=x_nm[:], in_=x_nm_d[:])
            txT = [pers.tile([128, LANES], BF16, tag=f"txT{k}",
                             name=f"txT{k}") for k in range(4)]
            nc.sync.dma_start(out=txT[0][:], in_=x_fm[:])
            tx1_nm = pers.tile([128, LANES], BF16, tag="tx1_nm", name="tx1_nm")
            W_t = []          # W_t[i][k]: [cin, cout] bf16
            for i in range(4):
                tiles = []
                for k in range(i + 1):
                    wt = pers.tile([D, D], BF16, tag=f"W{i}{k}", name=f"W_t{i}{k}")
                    nc.sync.dma_start(out=wt[:], in_=W_d[i][k])
                    tiles.append(wt)
                W_t.append(tiles)
            onesb = pers.tile([D, D], BF16, tag="onesb", name="onesb_t")
            nc.sync.dma_start(out=onesb[:], in_=onesb_d[:])
            bias_t = []
            for i in range(4):
                bt = pers.tile([D, D], BF16, tag=f"bias{i}", name=f"bias_t{i}")
                nc.sync.dma_start(out=bt[:], in_=bias_d[i][:])
                bias_t.append(bt)
            ident = pers.tile([128, 128], BF16, tag="ident", name="ident")
            from concourse.masks import make_identity
            make_identity(nc, ident[:])
            zero_b = pers.tile([128, D], BF16, tag="zerob", name="zero_b")
            nc.gpsimd.memset(zero_b[:], 0.0)

            # DRAM: AG bounces + replicas
            bounce = [dram.tile([SHARD_ROWS, D], BF16, tag=f"bounce{k}",
                                name=f"bounce{k}") for k in range(2)]
            repl = [dram.tile([TOT_ROWS, D], BF16, tag=f"repl{k}",
                              name=f"repl{k}", addr_space="Shared")
                    for k in range(2)]
            for k in range(2):
                nc.sync.dma_start(out=bounce[k][SHARD_ROWS - 1:SHARD_ROWS, :],
                                  in_=zero_b[0:1, :])

            def dense_tile(i, g):
                ps = pd.tile([128, 128], F32, tag="pdt", name="pdt")
                nc.tensor.matmul(out=ps[:], lhsT=onesb[:], rhs=bias_t[i][:],
                                 start=True, stop=False)
                for k in range(i + 1):
                    nc.tensor.matmul(out=ps[:],
                                     lhsT=txT[k][:, g * 128:(g + 1) * 128],
                                     rhs=W_t[i][k][:],
                                     start=False, stop=(k == i))
                ot = outp.tile([128, D], F32, tag="ot", name="ot")
                nc.scalar.activation(out=ot[:], in_=ps[:],
                                     func=mybir.ActivationFunctionType.Relu)
                nc.sync.dma_start(out=out_d[i][g * 128:(g + 1) * 128, :],
                                  in_=ot[:])

            for g in range(G):
                dense_tile(0, g)

            def stage(k):
                """k = 0,1,2 computes Tx_{k+1}; gathers from src replica."""
                src = xr if k == 0 else repl[k - 1]
                winA = src[WIN_A[0]:WIN_A[1], :]
                winB = src[WIN_B[0]:WIN_B[1], :]
                wk = w_t[k]
                for bi, info in enumerate(calls):
                    gs = info["groups"]
                    nchA, nchB = info["nch0"], info["nch1"]
                    nch = nchA + nchB
                    m = msgs_pool.tile([128, nch, D], BF16, tag="m", name="m")
                    for w, win in ((0, winA), (1, winB)):
                        nw = info[f"nch{w}"]
                        if nw == 0:
                            continue
                        o0 = 0 if w == 0 else nchA
                        # queue 0 desc-gen runs synchronously on the Pool
                        # engine; queues >= 1 generate asynchronously on a
                        # Q7 worker — the engine retires in ~1-2us
                        nc.gpsimd.dma_gather(
                            out_ap=m[:, o0:o0 + nw, :],
                            in_ap=win,
                            idxs_ap=idx_t[:, info[f"coff{w}"]:
                                          info[f"coff{w}"] + nw * 8],
                            num_idxs=nw * 128,
                            num_idxs_reg=nw * 128,
                            elem_size=D,
                            single_packet=False,
                            queue_num=1 + w,
                        )
                        # scale by w (broadcast along feat)
                        nc.vector.tensor_tensor(
                            out=m[:, o0:o0 + nw, :],
                            in0=m[:, o0:o0 + nw, :],
                            in1=wk[:, info[f"woff{w}"]:info[f"woff{w}"] + nw]
                                .unsqueeze(2).broadcast_to([128, nw, D]),
                            op=mybir.AluOpType.mult,
                        )
                    txb = txbp.tile([128, len(gs) * 128], BF16, tag="txb",
                                    name="txb")
                    for gi, g in enumerate(gs):
                        gsl = slice(g * 128, (g + 1) * 128)
                        s0, s1 = int(S0n[g]), int(S1n[g])
                        oA = info["offs0"][g]
                        oB = nchA + info["offs1"][g]
                        parts = [(oA, s0), (oB, s1)]
                        parts = [(o, s) for (o, s) in parts if s > 0]
                        assert parts
                        # segment-sum on the TensorEngine: accumulate the
                        # group's chunks into PSUM via identity matmuls
                        P = pr.tile([128, 128], F32, tag="pr", name="pr")
                        seq = [o + j for (o, s) in parts for j in range(s)]
                        for ji, c in enumerate(seq):
                            nc.tensor.matmul(out=P[:], lhsT=ident[:],
                                             rhs=m[:, c, :],
                                             start=(ji == 0),
                                             stop=(ji == len(seq) - 1))
                        # recurrence: Tx_{k+1} = A + B_SCALE * partner
                        if k > 0:
                            partner = x_nm if k == 1 else tx1_nm
                            nc.vector.scalar_tensor_tensor(
                                out=P[:],
                                in0=partner[:, gsl],
                                scalar=float(B_SCALE[k]),
                                in1=P[:],
                                op0=mybir.AluOpType.mult,
                                op1=mybir.AluOpType.add,
                            )
                        # bf16 node-major copy (bounce staging + transpose src)
                        nc.scalar.copy(out=txb[:, gi * 128:(gi + 1) * 128],
                                       in_=P[:])
                        if k == 0:
                            nc.scalar.copy(out=tx1_nm[:, gsl],
                                           in_=txb[:, gi * 128:(gi + 1) * 128])
                        psT = pt.tile([128, 128], BF16, tag="ptt", name="ptt")
                        nc.tensor.transpose(
                            out=psT[:],
                            in_=txb[:, gi * 128:(gi + 1) * 128],
                            identity=ident[:])
                        nc.scalar.copy(out=txT[k + 1][:, gsl], in_=psT[:])
                        dense_tile(k + 1, g)
                    if k < 2:
                        r0 = info["lane0"]
                        nr = info["nlanes"]
                        nc.sync.dma_start(
                            out=bounce[k][r0:r0 + nr, :]
                                .rearrange("(j p) f -> p j f", p=128),
                            in_=txb[:].rearrange("p (j f) -> p j f", f=D))
                if k < 2:
                    nc.gpsimd.collective_compute(
                        "AllGather",
                        mybir.AluOpType.bypass,
                        replica_groups=[list(range(NCORES))],
                        ins=[bounce[k][:].opt()],
                        outs=[repl[k][:].opt()],
                    )

            stage(0)
            stage(1)
            stage(2)

    nc.compile()
    return nc


def kernel(x, edge_index, edge_weight, W1, W2, W3, W4, b1, b2, b3, b4,
           _trace=False):
    import ml_dtypes
    x = np.asarray(x, np.float32)
    edge_index = np.asarray(edge_index)
    edge_weight = np.asarray(edge_weight, np.float32)
    Ws = [np.asarray(w, np.float32) for w in (W1, W2, W3, W4)]
    bs = [np.asarray(b, np.float32) for b in (b1, b2, b3, b4)]

    idx_arrs, w_base, perm_row, meta = _preprocess(edge_index, edge_weight)
    nc = _build_program(meta)

    # replica of x in permuted layout (zero rows stay zero)
    xr = np.zeros((TOT_ROWS, D), np.float32)
    xr[perm_row] = x
    xr = xr.astype(ml_dtypes.bfloat16)
    onesb = np.zeros((D, D), np.float32); onesb[0, :] = 1.0
    in_maps = []
    for c in range(NCORES):
        nol = meta["node_of_lane"][c]
        xs_c = np.zeros((LANES, D), np.float32)
        real = nol >= 0
        xs_c[real] = x[nol[real]]
        xs_b = xs_c.astype(ml_dtypes.bfloat16)
        m = {
            "xr": xr,
            "x_fm": np.ascontiguousarray(xs_b.T),
            "x_nm": np.ascontiguousarray(
                xs_b.reshape(G, 128, D).transpose(1, 0, 2).reshape(128, G * D)),
            "idx": idx_arrs[c],
            "onesb": onesb.astype(ml_dtypes.bfloat16),
        }
        for k in range(3):
            m[f"w{k}"] = (A_SCALE[k] * w_base[c]).astype(ml_dtypes.bfloat16)
        for i in range(4):
            m[f"W{i+1}"] = Ws[i].astype(ml_dtypes.bfloat16)
            bb = np.zeros((D, D), np.float32); bb[0, :] = bs[i]
            m[f"bias{i+1}"] = bb.astype(ml_dtypes.bfloat16)
        in_maps.append(m)

    res = bass_utils.run_bass_kernel_spmd(
        nc, in_maps, core_ids=list(range(NCORES)), trace=_trace)

    outs = []
    for i in range(4):
        full = np.zeros((N, D), np.float32)
        for c in range(NCORES):
            nol = meta["node_of_lane"][c]
            real = nol >= 0
            full[nol[real]] = res.results[c][f"o{i+1}"][real]
        outs.append(full)
    if _trace:
        return tuple(outs), res
    return tuple(outs)


# revision 26
# speedup vs baseline: 1.1006x; 1.0776x over previous
"""Trainium2 Bass kernel for nn_CascadeLayer (gnn_message_passing).

Cascade of GegConv layers, K = 1..4, alpha = 0.5, lambda_max = 2.0.
Since 2/lambda_max == 1, lhat(h) == -prop(h), so the whole cascade is
three sparse propagates plus small dense matmuls:

    P1 = prop(x)    Tx1 = -P1
    P2 = prop(Tx1)  Tx2 = -1.5*P2 - 0.5*x
    P3 = prop(Tx2)  Tx3 = -(5/3)*P3 - (2/3)*Tx1
    out_i = relu(sum_k Tx_k @ W_i[k] + b_i)

Distribution: nodes sharded across 8 cores by range (graph parallel).
Edges are routed to the core owning their destination. Every edge goes
through the gather path: per-edge 256B bf16 source rows are fetched with
dma_gather from a local DRAM replica of the full activation (kept in
sync with an AllGather between stages), scaled by the precomputed
symmetric norm on the VectorEngine, and segment-summed per destination
via an ELL layout (lane=partition, slot=chunk) with a strided
tensor_reduce. Lanes are degree-sorted into 128-lane groups; groups are
packed into NB contiguous batches with balanced slot counts, and each
batch issues one big gather per source window (A = cores 0-4, B =
cores 3-7; two windows keep replica rows int16-addressable).
"""

import numpy as np

import concourse.bass as bass
import concourse.bacc as bacc
import concourse.mybir as mybir
import concourse.tile as tile
from concourse import bass_utils

NCORES = 8
N = 50000
E = 800000
D = 128
SH = 6250                 # real nodes per core shard
LANES = 6272              # padded lanes per core (49 * 128)
G = LANES // 128          # 49 lane groups
SHARD_ROWS = LANES + 1    # + zero row for padding gathers
TOT_ROWS = NCORES * SHARD_ROWS          # 50184 replica rows
WIN_A = (0, 5 * SHARD_ROWS)             # replica rows of cores 0-4 (31365)
WIN_B = (3 * SHARD_ROWS, TOT_ROWS)      # replica rows of cores 3-7 (31365)
A_SCALE = (-1.0, -1.5, -5.0 / 3.0)      # prop scale folded into w per stage
B_SCALE = (0.0, -0.5, -2.0 / 3.0)       # partner scale per stage
NB = 10                   # balanced batches per stage

F32 = mybir.dt.float32
BF16 = mybir.dt.bfloat16
I16 = mybir.dt.int16


def _preprocess(edge_index, edge_weight):
    """Build per-core gather/scale structures."""
    row = edge_index[0].astype(np.int64)
    col = edge_index[1].astype(np.int64)
    ew = np.asarray(edge_weight, np.float32)

    deg = np.zeros(N, np.float32)
    np.add.at(deg, row, ew)
    dis = np.where(deg > 0, 1.0 / np.sqrt(np.where(deg > 0, deg, 1.0)), 0.0)
    dis = dis.astype(np.float32)
    norm = (dis[row].astype(np.float64) * ew * dis[col]).astype(np.float32)

    core_of_node = np.minimum(np.arange(N) // SH, NCORES - 1)
    src_core = core_of_node[row]
    dst_core = core_of_node[col]

    # --- per-core lane ordering (degree snake-sort) -----------------------
    # hard0: src core 0-2 (must use window A), hard1: src core 5-7 (B),
    # free: src core 3-4 (either window).
    nodes_sorted = np.zeros((NCORES, LANES), np.int64)
    per_core = []
    lane_lo = np.zeros((NCORES, LANES), np.int64)   # hard0 count
    lane_hi = np.zeros((NCORES, LANES), np.int64)   # hard0 + free
    lane_tot = np.zeros((NCORES, LANES), np.int64)
    for c in range(NCORES):
        em = dst_core == c
        d_loc = col[em] - c * SH
        sc = src_core[em]
        h0 = np.bincount(d_loc[sc <= 2], minlength=SH)
        h1 = np.bincount(d_loc[sc >= 5], minlength=SH)
        fr = np.bincount(d_loc[(sc == 3) | (sc == 4)], minlength=SH)
        tot = h0 + h1 + fr
        # extend with dummy lanes (degree 0)
        tot_e = np.concatenate([tot, np.zeros(LANES - SH, np.int64)])
        sk_e = np.concatenate([h0 - h1, np.zeros(LANES - SH, np.int64)])
        h0_e = np.concatenate([h0, np.zeros(LANES - SH, np.int64)])
        fr_e = np.concatenate([fr, np.zeros(LANES - SH, np.int64)])
        ids_e = np.concatenate([np.arange(SH, dtype=np.int64) + c * SH,
                                np.full(LANES - SH, -1, np.int64)])
        # snake-sort by (tot, +-skew)
        key = np.where(tot_e % 2 == 0, sk_e, -sk_e)
        o = np.lexsort((key, tot_e))
        nodes_sorted[c] = ids_e[o]
        lane_lo[c] = h0_e[o]
        lane_hi[c] = h0_e[o] + fr_e[o]
        lane_tot[c] = tot_e[o]
        per_core.append(em)

    # joint capacity choice per (sorted-order) group: same for all cores
    S0g = np.zeros(G, np.int64)
    S1g = np.zeros(G, np.int64)
    for g in range(G):
        sl = slice(g * 128, (g + 1) * 128)
        LO = lane_lo[:, sl]; HI = lane_hi[:, sl]; T = lane_tot[:, sl]
        smin = int(LO.max())
        best = None
        for s0 in range(smin, smin + 64):
            s1 = int((T - np.minimum(HI, s0)).max())
            if best is None or s0 + s1 < best[0]:
                best = (s0 + s1, s0, s1)
        S0g[g], S1g[g] = best[1], best[2]

    # --- balanced batch assignment (LPT), then relabel lanes --------------
    sizes = S0g + S1g
    order = np.argsort(sizes)[::-1]
    cap = [-(-G // NB)] * NB
    batch_groups = [[] for _ in range(NB)]
    batch_load = np.zeros(NB, np.int64)
    for g in order:
        cands = [b for b in range(NB) if len(batch_groups[b]) < cap[b]]
        b = min(cands, key=lambda bb: batch_load[bb])
        batch_groups[b].append(int(g))
        batch_load[b] += sizes[g]
    new_order = [g for b in range(NB) for g in batch_groups[b]]
    # permute per-group arrays and lane-level arrays to the new order
    lane_perm = np.concatenate([np.arange(g * 128, (g + 1) * 128)
                                for g in new_order])
    S0n = S0g[new_order]
    S1n = S1g[new_order]
    nodes_sorted = nodes_sorted[:, lane_perm]
    lane_lo = lane_lo[:, lane_perm]
    lane_hi = lane_hi[:, lane_perm]
    lane_tot = lane_tot[:, lane_perm]

    lane_of_node = np.full(N, -1, np.int64)
    node_of_lane = np.full((NCORES, LANES), -1, np.int64)
    perm_row = np.zeros(N, np.int64)
    for c in range(NCORES):
        nodes = nodes_sorted[c]
        node_of_lane[c] = nodes
        real = nodes >= 0
        lane_of_node[nodes[real]] = np.nonzero(real)[0]
        perm_row[nodes[real]] = c * SHARD_ROWS + np.nonzero(real)[0]

    # per-core per-lane split: x = lane's window-A count
    xA = np.maximum(lane_lo, lane_tot - np.repeat(S1n, 128)[None, :])

    # --- batch/call structure (static, same for all cores) ----------------
    # global chunk sequence: [b0.A | b0.B | b1.A | b1.B | ...]
    batches = []
    g0 = 0
    for b in range(NB):
        nbg = len(batch_groups[b])
        batches.append(list(range(g0, g0 + nbg)))
        g0 += nbg
    calls = []          # per batch: dict with A/B info
    col_off = 0
    w_off = 0
    chunk_base = np.zeros((G, 2), np.int64)
    for bi, gs in enumerate(batches):
        info = {"groups": gs, "lane0": gs[0] * 128,
                "nlanes": len(gs) * 128}
        for w, Sn in ((0, S0n), (1, S1n)):
            nch = int(sum(Sn[g] for g in gs))
            o = 0
            offs = {}
            for g in gs:
                offs[g] = o
                chunk_base[g, w] = w_off + o
                o += int(Sn[g])
            info[f"nch{w}"] = nch
            info[f"coff{w}"] = col_off
            info[f"woff{w}"] = w_off
            info[f"offs{w}"] = offs
            col_off += nch * 128 // 16
            w_off += nch
        calls.append(info)
    total_cols = col_off
    total_chunks = w_off

    # --- fill idx / w slot arrays per core --------------------------------
    idx_arrs = np.zeros((NCORES, 128, total_cols), np.int16)
    w_base = np.zeros((NCORES, 128, total_chunks), np.float32)
    for c in range(NCORES):
        em = per_core[c]
        e_lane = lane_of_node[col[em]]
        e_src_row = perm_row[row[em]]
        e_norm = norm[em]
        e_free = (src_core[em] == 3) | (src_core[em] == 4)
        e_hard1 = src_core[em] >= 5
        # order edges per lane: hard0 first, then free, then hard1
        cls = np.where(e_hard1, 2, np.where(e_free, 1, 0))
        eo = np.lexsort((cls, e_lane))
        e_lane, e_src_row, e_norm = e_lane[eo], e_src_row[eo], e_norm[eo]
        # position within lane
        lane_start = np.zeros(LANES + 1, np.int64)
        np.add.at(lane_start, e_lane + 1, 1)
        lane_start = np.cumsum(lane_start)
        pos_in_lane = np.arange(len(e_lane)) - lane_start[e_lane]
        # window: first xA[lane] edges -> A, rest -> B
        in_a = pos_in_lane < xA[c][e_lane]
        slot = np.where(in_a, pos_in_lane, pos_in_lane - xA[c][e_lane])
        widx = np.where(in_a, e_src_row - WIN_A[0], e_src_row - WIN_B[0])
        assert widx.min() >= 0
        assert widx.max() < 32768

        e_grp = e_lane // 128
        e_li = e_lane % 128
        e_chunk = chunk_base[e_grp, np.where(in_a, 0, 1)] + slot
        flat_idx = np.full(total_chunks * 128, -1, np.int64)
        flat_w = np.zeros(total_chunks * 128, np.float32)
        p = e_chunk * 128 + e_li
        assert len(np.unique(p)) == len(p)
        flat_idx[p] = widx
        flat_w[p] = e_norm
        # pads -> zero row of the window (row 6272 relative to window base)
        flat = flat_idx.reshape(total_chunks, 128)
        flat[flat < 0] = SHARD_ROWS - 1
        # wrapped layout: wrapped[q, col] = L[col*16 + q]
        wrapped = flat_idx.reshape(total_cols, 16).T.astype(np.int16)
        idx_arrs[c] = np.tile(wrapped, (8, 1))
        w_base[c] = flat_w.reshape(total_chunks, 128).T  # [li, chunk]

    meta = dict(S0n=S0n, S1n=S1n, calls=calls, batches=batches,
                total_cols=total_cols, total_chunks=total_chunks,
                node_of_lane=node_of_lane)
    return idx_arrs, w_base, perm_row, meta


def _build_program(meta):
    total_cols = meta["total_cols"]
    total_chunks = meta["total_chunks"]
    calls = meta["calls"]
    S0n, S1n = meta["S0n"], meta["S1n"]

    nc = bacc.Bacc("TRN2", target_bir_lowering=False, debug=False,
                   num_devices=NCORES, num_swdge_queues=4)

    xr = nc.dram_tensor("xr", [TOT_ROWS, D], BF16, kind="ExternalInput")
    x_fm = nc.dram_tensor("x_fm", [128, LANES], BF16, kind="ExternalInput")
    x_nm_d = nc.dram_tensor("x_nm", [128, LANES], BF16, kind="ExternalInput")
    idx_d = nc.dram_tensor("idx", [128, total_cols], I16, kind="ExternalInput")
    w_d = [nc.dram_tensor(f"w{k}", [128, total_chunks], BF16, kind="ExternalInput")
           for k in range(3)]
    W_d = [nc.dram_tensor(f"W{i+1}", [i + 1, D, D], BF16, kind="ExternalInput")
           for i in range(4)]
    onesb_d = nc.dram_tensor("onesb", [D, D], BF16, kind="ExternalInput")
    bias_d = [nc.dram_tensor(f"bias{i+1}", [D, D], BF16, kind="ExternalInput")
              for i in range(4)]
    out_d = [nc.dram_tensor(f"o{i+1}", [LANES, D], F32, kind="ExternalOutput")
             for i in range(4)]

    with tile.TileContext(nc) as tc:
        with (
            tc.tile_pool(name="pers", bufs=1) as pers,
            tc.tile_pool(name="msgs", bufs=3) as msgs_pool,
            tc.tile_pool(name="work", bufs=4) as work,
            tc.tile_pool(name="txbp", bufs=2) as txbp,
            tc.tile_pool(name="outp", bufs=3) as outp,
            tc.tile_pool(name="pt", bufs=2, space="PSUM") as pt,
            tc.tile_pool(name="pd", bufs=2, space="PSUM") as pd,
            tc.tile_pool(name="pr", bufs=4, space="PSUM") as pr,
            tc.tile_pool(name="dram", bufs=1, space="DRAM") as dram,
        ):
            # ---------------- prologue ----------------
            idx_t = pers.tile([128, total_cols], I16, tag="idx", name="idx_t")
            nc.sync.dma_start(out=idx_t[:], in_=idx_d[:])
            w_t = [pers.tile([128, total_chunks], BF16, tag=f"w{k}",
                             name=f"w_t{k}") for k in range(3)]
            for k in range(3):
                nc.sync.dma_start(out=w_t[k][:], in_=w_d[k][:])
            x_nm = pers.tile([128, LANES], BF16, tag="x_nm", name="x_nm")
            nc.sync.dma_start(out=x_nm[:], in_=x_nm_d[:])
            txT = [pers.tile([128, LANES], BF16, tag=f"txT{k}",
                             name=f"txT{k}") for k in range(4)]
            nc.sync.dma_start(out=txT[0][:], in_=x_fm[:])
            tx1_nm = pers.tile([128, LANES], BF16, tag="tx1_nm", name="tx1_nm")
            W_t = []          # W_t[i][k]: [cin, cout] bf16
            for i in range(4):
                tiles = []
                for k in range(i + 1):
                    wt = pers.tile([D, D], BF16, tag=f"W{i}{k}", name=f"W_t{i}{k}")
                    nc.sync.dma_start(out=wt[:], in_=W_d[i][k])
                    tiles.append(wt)
                W_t.append(tiles)
            onesb = pers.tile([D, D], BF16, tag="onesb", name="onesb_t")
            nc.sync.dma_start(out=onesb[:], in_=onesb_d[:])
            bias_t = []
            for i in range(4):
                bt = pers.tile([D, D], BF16, tag=f"bias{i}", name=f"bias_t{i}")
                nc.sync.dma_start(out=bt[:], in_=bias_d[i][:])
                bias_t.append(bt)
            ident = pers.tile([128, 128], BF16, tag="ident", name="ident")
            from concourse.masks import make_identity
            make_identity(nc, ident[:])
            # B_SCALE-scaled identities: fold the recurrence partner add
            # into the PE accumulation (P += B_SCALE * partner)
            identB = {}
            for k in (1, 2):
                ib = pers.tile([128, 128], BF16, tag=f"identB{k}",
                               name=f"identB{k}")
                nc.scalar.mul(out=ib[:], in_=ident[:], mul=float(B_SCALE[k]))
                identB[k] = ib
            zero_b = pers.tile([128, D], BF16, tag="zerob", name="zero_b")
            nc.gpsimd.memset(zero_b[:], 0.0)

            # DRAM: AG bounces + replicas
            bounce = [dram.tile([SHARD_ROWS, D], BF16, tag=f"bounce{k}",
                                name=f"bounce{k}") for k in range(2)]
            repl = [dram.tile([TOT_ROWS, D], BF16, tag=f"repl{k}",
                              name=f"repl{k}", addr_space="Shared")
                    for k in range(2)]
            for k in range(2):
                nc.sync.dma_start(out=bounce[k][SHARD_ROWS - 1:SHARD_ROWS, :],
                                  in_=zero_b[0:1, :])

            def dense_tile(i, g):
                ps = pd.tile([128, 128], F32, tag="pdt", name="pdt")
                nc.tensor.matmul(out=ps[:], lhsT=onesb[:], rhs=bias_t[i][:],
                                 start=True, stop=False)
                for k in range(i + 1):
                    nc.tensor.matmul(out=ps[:],
                                     lhsT=txT[k][:, g * 128:(g + 1) * 128],
                                     rhs=W_t[i][k][:],
                                     start=False, stop=(k == i))
                ot = outp.tile([128, D], F32, tag="ot", name="ot")
                nc.scalar.activation(out=ot[:], in_=ps[:],
                                     func=mybir.ActivationFunctionType.Relu)
                nc.sync.dma_start(out=out_d[i][g * 128:(g + 1) * 128, :],
                                  in_=ot[:])

            for g in range(G):
                dense_tile(0, g)

            def stage(k):
                """k = 0,1,2 computes Tx_{k+1}; gathers from src replica."""
                src = xr if k == 0 else repl[k - 1]
                winA = src[WIN_A[0]:WIN_A[1], :]
                winB = src[WIN_B[0]:WIN_B[1], :]
                wk = w_t[k]
                for bi, info in enumerate(calls):
                    gs = info["groups"]
                    nchA, nchB = info["nch0"], info["nch1"]
                    nch = nchA + nchB
                    m = msgs_pool.tile([128, nch, D], BF16, tag="m", name="m")
                    for w, win in ((0, winA), (1, winB)):
                        nw = info[f"nch{w}"]
                        if nw == 0:
                            continue
                        o0 = 0 if w == 0 else nchA
                        # split across SWDGE queues 1/2: queue-0 desc-gen
                        # runs synchronously on the Pool engine, queues >= 1
                        # generate asynchronously on a Q7 worker
                        h0 = nw // 2
                        for q, c0, cn in ((1, 0, h0), (2, h0, nw - h0)):
                            if cn == 0:
                                continue
                            nc.gpsimd.dma_gather(
                                out_ap=m[:, o0 + c0:o0 + c0 + cn, :],
                                in_ap=win,
                                idxs_ap=idx_t[:, info[f"coff{w}"] + c0 * 8:
                                              info[f"coff{w}"] + (c0 + cn) * 8],
                                num_idxs=cn * 128,
                                num_idxs_reg=cn * 128,
                                elem_size=D,
                                single_packet=False,
                                queue_num=q,
                            )
                        # scale by w (broadcast along feat)
                        nc.vector.tensor_tensor(
                            out=m[:, o0:o0 + nw, :],
                            in0=m[:, o0:o0 + nw, :],
                            in1=wk[:, info[f"woff{w}"]:info[f"woff{w}"] + nw]
                                .unsqueeze(2).broadcast_to([128, nw, D]),
                            op=mybir.AluOpType.mult,
                        )
                    txb = txbp.tile([128, len(gs) * 128], BF16, tag="txb",
                                    name="txb")
                    # pass 1: all groups' segment-sums on the TensorEngine
                    # (accumulating identity matmuls into PSUM, recurrence
                    # partner folded in via a scaled identity) — frees the
                    # m tile as early as possible
                    Pt = {}
                    for gi, g in enumerate(gs):
                        gsl = slice(g * 128, (g + 1) * 128)
                        s0, s1 = int(S0n[g]), int(S1n[g])
                        oA = info["offs0"][g]
                        oB = nchA + info["offs1"][g]
                        parts = [(oA, s0), (oB, s1)]
                        parts = [(o, s) for (o, s) in parts if s > 0]
                        assert parts
                        P = pr.tile([128, 128], F32, tag="pr", name="pr")
                        Pt[g] = P
                        seq = [o + j for (o, s) in parts for j in range(s)]
                        for ji, c in enumerate(seq):
                            nc.tensor.matmul(out=P[:], lhsT=ident[:],
                                             rhs=m[:, c, :],
                                             start=(ji == 0),
                                             stop=(k == 0 and
                                                   ji == len(seq) - 1))
                        if k > 0:
                            partner = x_nm if k == 1 else tx1_nm
                            nc.tensor.matmul(out=P[:], lhsT=identB[k][:],
                                             rhs=partner[:, gsl],
                                             start=False, stop=True)
                    # pass 2: per-group epilogue (bounce staging, transpose
                    # into the feature-major basis, output tiles)
                    for gi, g in enumerate(gs):
                        gsl = slice(g * 128, (g + 1) * 128)
                        P = Pt[g]
                        nc.scalar.copy(out=txb[:, gi * 128:(gi + 1) * 128],
                                       in_=P[:])
                        if k == 0:
                            nc.scalar.copy(out=tx1_nm[:, gsl],
                                           in_=txb[:, gi * 128:(gi + 1) * 128])
                        psT = pt.tile([128, 128], BF16, tag="ptt", name="ptt")
                        nc.tensor.transpose(
                            out=psT[:],
                            in_=txb[:, gi * 128:(gi + 1) * 128],
                            identity=ident[:])
                        nc.scalar.copy(out=txT[k + 1][:, gsl], in_=psT[:])
                        dense_tile(k + 1, g)
                    if k < 2:
                        r0 = info["lane0"]
                        nr = info["nlanes"]
                        nc.sync.dma_start(
                            out=bounce[k][r0:r0 + nr, :]
                                .rearrange("(j p) f -> p j f", p=128),
                            in_=txb[:].rearrange("p (j f) -> p j f", f=D))
                if k < 2:
                    nc.gpsimd.collective_compute(
                        "AllGather",
                        mybir.AluOpType.bypass,
                        replica_groups=[list(range(NCORES))],
                        ins=[bounce[k][:].opt()],
                        outs=[repl[k][:].opt()],
                    )

            stage(0)
            stage(1)
            stage(2)

    nc.compile()
    return nc


def kernel(x, edge_index, edge_weight, W1, W2, W3, W4, b1, b2, b3, b4,
           _trace=False):
    import ml_dtypes
    x = np.asarray(x, np.float32)
    edge_index = np.asarray(edge_index)
    edge_weight = np.asarray(edge_weight, np.float32)
    Ws = [np.asarray(w, np.float32) for w in (W1, W2, W3, W4)]
    bs = [np.asarray(b, np.float32) for b in (b1, b2, b3, b4)]

    idx_arrs, w_base, perm_row, meta = _preprocess(edge_index, edge_weight)
    nc = _build_program(meta)

    # replica of x in permuted layout (zero rows stay zero)
    xr = np.zeros((TOT_ROWS, D), np.float32)
    xr[perm_row] = x
    xr = xr.astype(ml_dtypes.bfloat16)
    onesb = np.zeros((D, D), np.float32); onesb[0, :] = 1.0
    in_maps = []
    for c in range(NCORES):
        nol = meta["node_of_lane"][c]
        xs_c = np.zeros((LANES, D), np.float32)
        real = nol >= 0
        xs_c[real] = x[nol[real]]
        xs_b = xs_c.astype(ml_dtypes.bfloat16)
        m = {
            "xr": xr,
            "x_fm": np.ascontiguousarray(xs_b.T),
            "x_nm": np.ascontiguousarray(
                xs_b.reshape(G, 128, D).transpose(1, 0, 2).reshape(128, G * D)),
            "idx": idx_arrs[c],
            "onesb": onesb.astype(ml_dtypes.bfloat16),
        }
        for k in range(3):
            m[f"w{k}"] = (A_SCALE[k] * w_base[c]).astype(ml_dtypes.bfloat16)
        for i in range(4):
            m[f"W{i+1}"] = Ws[i].astype(ml_dtypes.bfloat16)
            bb = np.zeros((D, D), np.float32); bb[0, :] = bs[i]
            m[f"bias{i+1}"] = bb.astype(ml_dtypes.bfloat16)
        in_maps.append(m)

    res = bass_utils.run_bass_kernel_spmd(
        nc, in_maps, core_ids=list(range(NCORES)), trace=_trace)

    outs = []
    for i in range(4):
        full = np.zeros((N, D), np.float32)
        for c in range(NCORES):
            nol = meta["node_of_lane"][c]
            real = nol >= 0
            full[nol[real]] = res.results[c][f"o{i+1}"][real]
        outs.append(full)
    if _trace:
        return tuple(outs), res
    return tuple(outs)
